# revision 5
# baseline (speedup 1.0000x reference)
"""Trainium2 Bass kernel for nn_DebiasIntraDist (segment_reduce).

Full-input contract: kernel(**inputs) takes the complete (unsharded) inputs
and returns the full scalar loss. Sharding: core 2d+h gets the rows with
demog == d and label-half h, so every core owns a disjoint set of 256
(demog, label) groups. Within a core, rows are partitioned into two
segments by label-quarter (local label < 128 vs >= 128), each padded to a
whole number of 128-row tiles (T0, T1 known at compile time). Every tile
therefore feeds exactly ONE 128-group PSUM accumulator - half the matmul
work of an unsorted layout.

Per 128-row tile (single pass over feats, fp32r matmuls at bf16 speed):
    oh   = one_hot(labels_local)            # vector, [128,128]
    xsq  = X * X                            # scalar engine, [128,512]
    sums[seg]  += oh^T @ X                  # tensor, fp32r
    sumsq[seg] += oh^T @ xsq                # tensor, fp32r
After each segment: norm2[g] = sum_d sums[g,d]^2 (vector ttr) and
sumsq_g[g] = sum_d sumsq[g,d] (vector reduce) -> out tile [128, 4].

Each core DMAs its [128, 4] partial stats out; the host (which already
knows the per-group counts from the shard step) finishes the tiny
O(G) reduction to the scalar loss in fp64. No collectives anywhere.
"""

import os
import numpy as np

try:
    import concourse.bacc as bacc
except ImportError:  # fresh environment without PYTHONPATH set up
    import sys
    for p in ("/root/.axon_site/_ro/trn_rl_repo", "/opt/trn_rl_repo",
              "/root/.axon_site/_ro/pypackages"):
        if p not in sys.path:
            sys.path.append(p)
    import concourse.bacc as bacc
import concourse.mybir as mybir
import concourse.tile as tile
import concourse.bass_utils as bass_utils

N_CORES = 8
P = 128
D = 512          # feature dim
ND = 4           # demog values
CH = 6           # sample-tiles per feats DMA (1.5 MiB)
PAD_LABEL = 500.0  # never matches iota [0,128)

MODE = os.environ.get("BASS_MODE", "hilo")  # "f32r" | "hilo"

_cache: dict[tuple, object] = {}


def _build(T0: int, T1: int, mode: str = "f32r"):
    """Compile the SPMD kernel: T0/T1 = tiles in segment 0/1."""
    T = T0 + T1
    S = T * P
    fp32 = mybir.dt.float32
    f32r = mybir.dt.float32r
    bf16 = mybir.dt.bfloat16
    Alu = mybir.AluOpType
    Act = mybir.ActivationFunctionType
    oh_dt = fp32 if mode == "f32r" else bf16

    nc = bacc.Bacc("TRN2", target_bir_lowering=False, debug=False,
                   enable_asserts=True, num_devices=N_CORES)

    feats = nc.dram_tensor("feats", [S, D], fp32, kind="ExternalInput").ap()
    labels_t = nc.dram_tensor("labels_t", [P, T], fp32, kind="ExternalInput").ap()
    part = nc.dram_tensor("part", [P, 4], fp32, kind="ExternalOutput").ap()

    with tile.TileContext(nc) as tc:
        with (
            tc.tile_pool(name="const", bufs=1) as constp,
            tc.tile_pool(name="fx", bufs=4) as fxp,
            tc.tile_pool(name="oh", bufs=8) as ohp,
            tc.tile_pool(name="sq", bufs=6) as sqp,
            tc.tile_pool(name="hl", bufs=6) as hlp,
            tc.tile_pool(name="post", bufs=1) as postp,
            tc.tile_pool(name="ps", bufs=1, space="PSUM") as psp,
        ):
            # constants
            labs = constp.tile([P, T], fp32, tag="labs")
            nc.scalar.dma_start(out=labs[:], in_=labels_t[:])
            iota = constp.tile([P, P], oh_dt, tag="iota")
            nc.gpsimd.iota(iota[:], [[1, P]], channel_multiplier=0,
                           allow_small_or_imprecise_dtypes=True)

            # per-segment accumulators: one PSUM bank each
            ps_sums = [psp.tile([P, D], fp32, tag=f"sums{s}", name=f"sums{s}")
                       for s in range(2)]
            ps_sq = [psp.tile([P, D], fp32, tag=f"sq{s}", name=f"sq{s}")
                     for s in range(2)]
            out4 = postp.tile([P, 4], fp32, tag="out4")

            feats_r = feats.rearrange("(n p) d -> n p d", p=P)  # [T, P, D]

            def seg_post(seg):
                # norm2[g] = sum_d sums[g,d]^2 ; sumsq_g[g] = sum_d sumsq[g,d]
                scr = postp.tile([P, D], fp32, tag=f"scr{seg}",
                                 name=f"scr{seg}")
                nc.scalar.activation(
                    scr[:], ps_sums[seg][:], Act.Square,
                    accum_out=out4[:, 2 * seg + 1:2 * seg + 2])
                nc.vector.tensor_reduce(
                    out=out4[:, 2 * seg:2 * seg + 1], in_=ps_sq[seg][:],
                    axis=mybir.AxisListType.X, op=Alu.add)

            t = 0
            first_chunk = True
            while t < T:
                # small first chunk so compute starts ASAP
                L = 1 if first_chunk else min(CH, T - t)
                first_chunk = False
                fx = fxp.tile([P, CH, D], fp32, tag="fx")
                nc.sync.dma_start(
                    out=fx[:, :L, :],
                    in_=feats_r[t:t + L].rearrange("n p d -> p n d"),
                )
                fxb = fx[:].bitcast(bf16)  # [P, CH, 2*D] for hi view
                for j in range(L):
                    ti = t + j
                    seg = 0 if ti < T0 else 1
                    st = ti in (0, T0)
                    sp = ti in (T0 - 1, T - 1)
                    X = fx[:, j, :]
                    oh = ohp.tile([P, P], oh_dt, tag="oh")
                    # one-hot on the otherwise-idle gpsimd engine; it runs
                    # far ahead of the loop so LDWEIGHTS always overlaps
                    nc.gpsimd.tensor_scalar(
                        out=oh[:], in0=iota[:], scalar1=labs[:, ti:ti + 1],
                        scalar2=None, op0=Alu.is_equal,
                    )
                    if mode == "f32r":
                        xsq = sqp.tile([P, D], fp32, tag="xsq")
                        nc.scalar.activation(xsq[:], X, Act.Square)
                        ohr = oh[:].bitcast(f32r)
                        nc.tensor.matmul(out=ps_sums[seg][:], lhsT=ohr,
                                         rhs=X.bitcast(f32r),
                                         start=st, stop=sp)
                        nc.tensor.matmul(out=ps_sq[seg][:], lhsT=ohr,
                                         rhs=xsq[:].bitcast(f32r),
                                         start=st, stop=sp)
                    else:
                        xhi = fxb[:, j, 1::2]  # truncated-bf16 view of X
                        xlo = hlp.tile([P, D], bf16, tag="xlo")
                        nc.vector.tensor_tensor(out=xlo[:], in0=X, in1=xhi,
                                                op=Alu.subtract)
                        xsq = sqp.tile([P, D], bf16, tag="xsq")
                        nc.scalar.activation(xsq[:], X, Act.Square)
                        nc.tensor.matmul(out=ps_sums[seg][:], lhsT=oh[:],
                                         rhs=xhi, start=st, stop=False)
                        nc.tensor.matmul(out=ps_sums[seg][:], lhsT=oh[:],
                                         rhs=xlo[:], start=False, stop=sp)
                        nc.tensor.matmul(out=ps_sq[seg][:], lhsT=oh[:],
                                         rhs=xsq[:], start=st, stop=sp)
                    if sp:
                        seg_post(seg)
                t += L

            nc.sync.dma_start(out=part[:], in_=out4[:])

    nc.compile()
    return nc


def _prepare(feats, labels, demog):
    """Shard rows by (demog, label-half); sort each shard into two
    label-quarter segments padded to whole tiles. Returns the compile key,
    per-core input maps, and per-(core, seg, slot) counts for host combine.
    """
    lab256 = labels % 256
    core_id = demog * 2 + (labels >= 256).astype(np.int32)
    seg_id = (lab256 >= 128).astype(np.int32)
    labloc = (lab256 % 128).astype(np.float32)

    idx = [[np.flatnonzero((core_id == k) & (seg_id == s)) for s in range(2)]
           for k in range(N_CORES)]
    T0 = max(1, max(-(-len(idx[k][0]) // P) for k in range(N_CORES)))
    T1 = max(1, max(-(-len(idx[k][1]) // P) for k in range(N_CORES)))
    T = T0 + T1
    S = T * P

    in_maps = []
    cnts = np.zeros((N_CORES, 2, P), np.int64)
    for k in range(N_CORES):
        f = np.zeros((S, D), np.float32)
        lab = np.full(S, PAD_LABEL, np.float32)
        for s, base in ((0, 0), (1, T0 * P)):
            rows = idx[k][s]
            f[base:base + len(rows)] = feats[rows]
            lab[base:base + len(rows)] = labloc[rows]
            cnts[k, s] = np.bincount(labloc[rows].astype(np.int64),
                                     minlength=P)
        labs_t = np.ascontiguousarray(lab.reshape(T, P).T)
        in_maps.append({"feats": f, "labels_t": labs_t})
    return (T0, T1), in_maps, cnts


def _combine(parts, cnts):
    """Finish the reduction on host in fp64: parts[k] = [128,4] device out."""
    num = np.zeros(ND, np.float64)
    den = np.zeros(ND, np.float64)
    for k in range(N_CORES):
        d = k // 2
        p = np.asarray(parts[k], np.float64)
        for s in range(2):
            ssg = p[:, 2 * s]          # per-group sum of ||x||^2
            nn2 = p[:, 2 * s + 1]      # per-group ||sums||^2
            c = cnts[k, s].astype(np.float64)
            safe = np.maximum(c, 1.0)
            grp = (ssg - nn2 / safe) / safe
            pres = (c > 0)
            num[d] += grp[pres].sum()
            den[d] += pres.sum()
    intra = num / np.maximum(den, 1.0)
    return np.float32(np.mean(np.abs(intra - intra.mean())))


def kernel(feats, labels, demog_labels, _results_out=None):
    feats = np.ascontiguousarray(np.asarray(feats), dtype=np.float32)
    labels = np.asarray(labels).astype(np.int32)
    demog = np.asarray(demog_labels).astype(np.int32)
    assert feats.ndim == 2 and feats.shape[1] == D

    key, in_maps, cnts = _prepare(feats, labels, demog)
    nc = _cache.get(key)
    if nc is None:
        nc = _cache.setdefault(key, _build(*key, mode=MODE))
    res = None
    last_exc = None
    for attempt in range(3):
        try:
            res = bass_utils.run_bass_kernel_spmd(
                nc, in_maps, core_ids=list(range(N_CORES)))
            break
        except Exception as e:  # transient axon worker hangups
            last_exc = e
            import time
            time.sleep(10)
    if res is None:
        raise last_exc
    if _results_out is not None:
        _results_out.append(res)
    return _combine([res.results[k]["part"] for k in range(N_CORES)], cnts)


# revision 7
# speedup vs baseline: 1.7772x; 1.7772x over previous
"""Trainium2 Bass kernel for nn_DebiasIntraDist (segment_reduce).

Full-input contract: kernel(**inputs) takes the complete (unsharded) inputs
and returns the full scalar loss. Sharding: core 2d+h gets the rows with
demog == d and label-half h, so every core owns a disjoint set of 256
(demog, label) groups. Within a core, rows are partitioned into two
segments by label-quarter (local label < 128 vs >= 128), each padded to a
whole number of 128-row tiles (T0, T1 known at compile time). Every tile
therefore feeds exactly ONE 128-group PSUM accumulator - half the matmul
work of an unsorted layout.

Per 128-row tile (single pass over feats, bf16 matmuls, error-compensated
hi/lo split so group sums are near-fp32 exact):
    oh   = one_hot(labels_local)  # vector IS_EQ, all-bf16 fast path
    xlo  = X - hi16(X)            # vector, bf16 out (hi16 = free strided view)
    xsq  = X * X                  # scalar engine Square, bf16 out
    sums[seg]  += oh^T @ hi + oh^T @ xlo    # tensor (one PSUM bank per seg)
    sumsq[seg] += oh^T @ xsq                # tensor
After each segment: norm2[g] = sum_d sums[g,d]^2 (scalar Square+accum) and
sumsq_g[g] = sum_d sumsq[g,d] (vector reduce) -> out tile [128, 4].

Feats are shipped partition-major ([P, T*D], rows t*128+p at column t*D)
so every DMA descriptor moves a contiguous 12 KiB run. iota and labels go
in as bf16 so the one-hot IS_EQ takes the 16-bit DVE fast path and the
loop can start as soon as the first feats tile lands - no gpsimd anywhere.

Each core DMAs its [128, 4] partial stats out; the host (which already
knows the per-group counts from the shard step) finishes the tiny O(G)
reduction to the scalar loss in fp64. No collectives anywhere.
"""

import os
import numpy as np
import ml_dtypes

try:
    import concourse.bacc as bacc
except ImportError:  # fresh environment without PYTHONPATH set up
    import sys
    for p in ("/root/.axon_site/_ro/trn_rl_repo", "/opt/trn_rl_repo",
              "/root/.axon_site/_ro/pypackages"):
        if p not in sys.path:
            sys.path.append(p)
    import concourse.bacc as bacc
import concourse.mybir as mybir
import concourse.tile as tile
import concourse.bass_utils as bass_utils

N_CORES = 8
P = 128
D = 512          # feature dim
ND = 4           # demog values
CH = 6           # sample-tiles per feats DMA (1.5 MiB)
PAD_LABEL = 500.0  # exact in bf16, never matches iota [0,128)

_cache: dict[tuple, object] = {}


def _build(T0: int, T1: int):
    """Compile the SPMD kernel: T0/T1 = tiles in segment 0/1."""
    T = T0 + T1
    fp32 = mybir.dt.float32
    bf16 = mybir.dt.bfloat16
    Alu = mybir.AluOpType
    Act = mybir.ActivationFunctionType

    nc = bacc.Bacc("TRN2", target_bir_lowering=False, debug=False,
                   enable_asserts=True, num_devices=N_CORES)

    feats = nc.dram_tensor("feats", [P, T * D], fp32,
                           kind="ExternalInput").ap()
    labels_t = nc.dram_tensor("labels_t", [P, T], fp32,
                              kind="ExternalInput").ap()
    iota_t = nc.dram_tensor("iota_t", [P, P], bf16,
                            kind="ExternalInput").ap()
    part = nc.dram_tensor("part", [P, 4], fp32, kind="ExternalOutput").ap()

    with tile.TileContext(nc) as tc:
        with (
            tc.tile_pool(name="const", bufs=1) as constp,
            tc.tile_pool(name="fx", bufs=4) as fxp,
            tc.tile_pool(name="oh", bufs=8) as ohp,
            tc.tile_pool(name="sq", bufs=6) as sqp,
            tc.tile_pool(name="hl", bufs=6) as hlp,
            tc.tile_pool(name="post", bufs=1) as postp,
            tc.tile_pool(name="ps", bufs=1, space="PSUM") as psp,
        ):
            # constants via DMA only - ready as soon as the queues go live
            labs = constp.tile([P, T], fp32, tag="labs")
            nc.scalar.dma_start(out=labs[:], in_=labels_t[:])
            iota = constp.tile([P, P], bf16, tag="iota")
            nc.scalar.dma_start(out=iota[:], in_=iota_t[:])

            # warm the activation table before the first real Square needs it
            warm = constp.tile([P, 1], fp32, tag="warm")
            nc.vector.memset(warm[:], 0.0)
            nc.scalar.activation(warm[:], warm[:], Act.Square)

            # per-segment accumulators: one PSUM bank each
            ps_sums = [psp.tile([P, D], fp32, tag=f"sums{s}", name=f"sums{s}")
                       for s in range(2)]
            ps_sq = [psp.tile([P, D], fp32, tag=f"sq{s}", name=f"sq{s}")
                     for s in range(2)]
            out4 = postp.tile([P, 4], fp32, tag="out4")

            def seg_post(seg):
                # norm2[g] = sum_d sums[g,d]^2 ; sumsq_g[g] = sum_d sumsq[g,d]
                scr = postp.tile([P, D], fp32, tag=f"scr{seg}",
                                 name=f"scr{seg}")
                nc.scalar.activation(
                    scr[:], ps_sums[seg][:], Act.Square,
                    accum_out=out4[:, 2 * seg + 1:2 * seg + 2])
                nc.vector.tensor_reduce(
                    out=out4[:, 2 * seg:2 * seg + 1], in_=ps_sq[seg][:],
                    axis=mybir.AxisListType.X, op=Alu.add)

            t = 0
            first_chunk = True
            while t < T:
                # small first chunk so compute starts ASAP
                L = 1 if first_chunk else min(CH, T - t)
                first_chunk = False
                fx = fxp.tile([P, CH * D], fp32, tag="fx")
                nc.sync.dma_start(out=fx[:, :L * D],
                                  in_=feats[:, t * D:(t + L) * D])
                fxb = fx[:].bitcast(bf16)  # [P, CH*2D] for the hi view
                for j in range(L):
                    ti = t + j
                    seg = 0 if ti < T0 else 1
                    st = ti in (0, T0)
                    sp = ti in (T0 - 1, T - 1)
                    X = fx[:, j * D:(j + 1) * D]
                    oh = ohp.tile([P, P], bf16, tag="oh")
                    nc.vector.tensor_scalar(
                        out=oh[:], in0=iota[:], scalar1=labs[:, ti:ti + 1],
                        scalar2=None, op0=Alu.is_equal,
                    )
                    xhi = fxb[:, 2 * j * D + 1:2 * (j + 1) * D:2]
                    xlo = hlp.tile([P, D], bf16, tag="xlo")
                    nc.vector.tensor_tensor(out=xlo[:], in0=X, in1=xhi,
                                            op=Alu.subtract)
                    xsq = sqp.tile([P, D], bf16, tag="xsq")
                    nc.scalar.activation(xsq[:], X, Act.Square)
                    nc.tensor.matmul(out=ps_sums[seg][:], lhsT=oh[:],
                                     rhs=xhi, start=st, stop=False)
                    nc.tensor.matmul(out=ps_sums[seg][:], lhsT=oh[:],
                                     rhs=xlo[:], start=False, stop=sp)
                    nc.tensor.matmul(out=ps_sq[seg][:], lhsT=oh[:],
                                     rhs=xsq[:], start=st, stop=sp)
                    if sp:
                        seg_post(seg)
                t += L

            nc.sync.dma_start(out=part[:], in_=out4[:])

    nc.compile()
    return nc


def _prepare(feats, labels, demog):
    """Shard rows by (demog, label-half); sort each shard into two
    label-quarter segments padded to whole tiles. Returns the compile key,
    per-core input maps, and per-(core, seg, slot) counts for host combine.
    """
    lab256 = labels % 256
    core_id = demog * 2 + (labels >= 256).astype(np.int32)
    seg_id = (lab256 >= 128).astype(np.int32)
    labloc = (lab256 % 128).astype(np.float32)

    idx = [[np.flatnonzero((core_id == k) & (seg_id == s)) for s in range(2)]
           for k in range(N_CORES)]
    T0 = max(1, max(-(-len(idx[k][0]) // P) for k in range(N_CORES)))
    T1 = max(1, max(-(-len(idx[k][1]) // P) for k in range(N_CORES)))
    T = T0 + T1
    S = T * P

    iota_np = np.broadcast_to(np.arange(P, dtype=np.float32), (P, P))
    iota_np = np.ascontiguousarray(iota_np).astype(ml_dtypes.bfloat16)
    in_maps = []
    cnts = np.zeros((N_CORES, 2, P), np.int64)
    for k in range(N_CORES):
        f = np.zeros((S, D), np.float32)
        lab = np.full(S, PAD_LABEL, np.float32)
        for s, base in ((0, 0), (1, T0 * P)):
            rows = idx[k][s]
            f[base:base + len(rows)] = feats[rows]
            lab[base:base + len(rows)] = labloc[rows]
            cnts[k, s] = np.bincount(labloc[rows].astype(np.int64),
                                     minlength=P)
        # partition-major: row t*128+p lives at f_t[p, t*D:(t+1)*D]
        f_t = np.ascontiguousarray(
            f.reshape(T, P, D).transpose(1, 0, 2).reshape(P, T * D))
        labs_t = np.ascontiguousarray(lab.reshape(T, P).T)
        in_maps.append({"feats": f_t, "labels_t": labs_t, "iota_t": iota_np})
    return (T0, T1), in_maps, cnts


def _combine(parts, cnts):
    """Finish the reduction on host in fp64: parts[k] = [128,4] device out."""
    num = np.zeros(ND, np.float64)
    den = np.zeros(ND, np.float64)
    for k in range(N_CORES):
        d = k // 2
        p = np.asarray(parts[k], np.float64)
        for s in range(2):
            ssg = p[:, 2 * s]          # per-group sum of ||x||^2
            nn2 = p[:, 2 * s + 1]      # per-group ||sums||^2
            c = cnts[k, s].astype(np.float64)
            safe = np.maximum(c, 1.0)
            grp = (ssg - nn2 / safe) / safe
            pres = (c > 0)
            num[d] += grp[pres].sum()
            den[d] += pres.sum()
    intra = num / np.maximum(den, 1.0)
    return np.float32(np.mean(np.abs(intra - intra.mean())))


def kernel(feats, labels, demog_labels, _results_out=None):
    feats = np.ascontiguousarray(np.asarray(feats), dtype=np.float32)
    labels = np.asarray(labels).astype(np.int32)
    demog = np.asarray(demog_labels).astype(np.int32)
    assert feats.ndim == 2 and feats.shape[1] == D

    key, in_maps, cnts = _prepare(feats, labels, demog)
    nc = _cache.get(key)
    if nc is None:
        nc = _cache.setdefault(key, _build(*key))
    res = None
    last_exc = None
    for attempt in range(3):
        try:
            res = bass_utils.run_bass_kernel_spmd(
                nc, in_maps, core_ids=list(range(N_CORES)))
            break
        except Exception as e:  # transient axon worker hangups
            last_exc = e
            import time
            time.sleep(10)
    if res is None:
        raise last_exc
    if _results_out is not None:
        _results_out.append(res)
    return _combine([res.results[k]["part"] for k in range(N_CORES)], cnts)


# revision 9
# speedup vs baseline: 2.2228x; 1.2507x over previous
"""Trainium2 Bass kernel for nn_DebiasIntraDist (segment_reduce).

Full-input contract: kernel(**inputs) takes the complete (unsharded) inputs
and returns the full scalar loss. Sharding: core 2d+h gets the rows with
demog == d and label-half h, so every core owns a disjoint set of 256
(demog, label) groups. Within a core, rows are partitioned into two
segments by label-quarter (local label < 128 vs >= 128), each padded to a
whole number of 128-row tiles (T0, T1 known at compile time). Every tile
therefore feeds exactly ONE 128-group PSUM accumulator - half the matmul
work of an unsorted layout.

Per 128-row tile (single pass over feats, bf16 matmuls, error-compensated
hi/lo split so group sums are near-fp32 exact):
    oh   = one_hot(labels_local)  # vector IS_EQ, all-bf16 fast path
    xlo  = X - hi16(X)            # vector, bf16 out (hi16 = free strided view)
    xsq  = X * X                  # scalar engine Square, bf16 out
    sums[seg]  += oh^T @ hi + oh^T @ xlo    # tensor (one PSUM bank per seg)
    sumsq[seg] += oh^T @ xsq                # tensor
After each segment: norm2[g] = sum_d sums[g,d]^2 (scalar Square+accum) and
sumsq_g[g] = sum_d sumsq[g,d] (vector reduce) -> out tile [128, 4].

Feats are shipped partition-major ([P, T*D], rows t*128+p at column t*D)
so every DMA descriptor moves a contiguous 12 KiB run. iota and labels go
in as bf16 so the one-hot IS_EQ takes the 16-bit DVE fast path and the
loop can start as soon as the first feats tile lands - no gpsimd anywhere.

Each core DMAs its [128, 4] partial stats out; the host (which already
knows the per-group counts from the shard step) finishes the tiny O(G)
reduction to the scalar loss in fp64. No collectives anywhere.
"""

import os
import numpy as np
import ml_dtypes

try:
    import concourse.bacc as bacc
except ImportError:  # fresh environment without PYTHONPATH set up
    import sys
    for p in ("/root/.axon_site/_ro/trn_rl_repo", "/opt/trn_rl_repo",
              "/root/.axon_site/_ro/pypackages"):
        if p not in sys.path:
            sys.path.append(p)
    import concourse.bacc as bacc
import concourse.mybir as mybir
import concourse.tile as tile
import concourse.bass_utils as bass_utils

N_CORES = 8
P = 128
D = 512          # feature dim
ND = 4           # demog values
CH = 6           # sample-tiles per feats DMA (1.5 MiB)
PAD_LABEL = 500.0  # exact in bf16, never matches iota [0,128)

_cache: dict[tuple, object] = {}


def _build(T0: int, T1: int):
    """Compile the SPMD kernel: T0/T1 = tiles in segment 0/1."""
    T = T0 + T1
    fp32 = mybir.dt.float32
    bf16 = mybir.dt.bfloat16
    Alu = mybir.AluOpType
    Act = mybir.ActivationFunctionType

    nc = bacc.Bacc("TRN2", target_bir_lowering=False, debug=False,
                   enable_asserts=True, num_devices=N_CORES)

    feats = nc.dram_tensor("feats", [T * P, D], fp32,
                           kind="ExternalInput").ap()
    labels_t = nc.dram_tensor("labels_t", [P, T], fp32,
                              kind="ExternalInput").ap()
    iota_t = nc.dram_tensor("iota_t", [P, P], bf16,
                            kind="ExternalInput").ap()
    part = nc.dram_tensor("part", [P, 4], fp32, kind="ExternalOutput").ap()

    with tile.TileContext(nc) as tc:
        with (
            tc.tile_pool(name="const", bufs=1) as constp,
            tc.tile_pool(name="fx", bufs=4) as fxp,
            tc.tile_pool(name="oh", bufs=8) as ohp,
            tc.tile_pool(name="sq", bufs=6) as sqp,
            tc.tile_pool(name="hl", bufs=6) as hlp,
            tc.tile_pool(name="post", bufs=1) as postp,
            tc.tile_pool(name="ps", bufs=1, space="PSUM") as psp,
        ):
            # constants via DMA only - ready as soon as the queues go live
            labs = constp.tile([P, T], fp32, tag="labs")
            nc.scalar.dma_start(out=labs[:], in_=labels_t[:])
            iota = constp.tile([P, P], bf16, tag="iota")
            nc.scalar.dma_start(out=iota[:], in_=iota_t[:])

            # warm the activation table before the first real Square needs it
            warm = constp.tile([P, 1], fp32, tag="warm")
            nc.vector.memset(warm[:], 0.0)
            nc.scalar.activation(warm[:], warm[:], Act.Square)

            # per-segment accumulators: one PSUM bank each
            ps_sums = [psp.tile([P, D], fp32, tag=f"sums{s}", name=f"sums{s}")
                       for s in range(2)]
            ps_sq = [psp.tile([P, D], fp32, tag=f"sq{s}", name=f"sq{s}")
                     for s in range(2)]
            out4 = postp.tile([P, 4], fp32, tag="out4")
            feats_r = feats.rearrange("(n p) d -> n p d", p=P)  # [T, P, D]

            def seg_post(seg):
                # norm2[g] = sum_d sums[g,d]^2 ; sumsq_g[g] = sum_d sumsq[g,d]
                scr = postp.tile([P, D], fp32, tag=f"scr{seg}",
                                 name=f"scr{seg}")
                nc.scalar.activation(
                    scr[:], ps_sums[seg][:], Act.Square,
                    accum_out=out4[:, 2 * seg + 1:2 * seg + 2])
                nc.vector.tensor_reduce(
                    out=out4[:, 2 * seg:2 * seg + 1], in_=ps_sq[seg][:],
                    axis=mybir.AxisListType.X, op=Alu.add)

            t = 0
            first_chunk = True
            while t < T:
                # small first chunk so compute starts ASAP
                L = 1 if first_chunk else min(CH, T - t)
                first_chunk = False
                fx = fxp.tile([P, CH, D], fp32, tag="fx")
                nc.sync.dma_start(
                    out=fx[:, :L, :],
                    in_=feats_r[t:t + L].rearrange("n p d -> p n d"))
                fxb = fx[:].bitcast(bf16)  # [P, CH, 2D] for the hi view
                for j in range(L):
                    ti = t + j
                    seg = 0 if ti < T0 else 1
                    st = ti in (0, T0)
                    sp = ti in (T0 - 1, T - 1)
                    X = fx[:, j, :]
                    oh = ohp.tile([P, P], bf16, tag="oh")
                    nc.vector.tensor_scalar(
                        out=oh[:], in0=iota[:], scalar1=labs[:, ti:ti + 1],
                        scalar2=None, op0=Alu.is_equal,
                    )
                    xhi = fxb[:, j, 1::2]
                    xlo = hlp.tile([P, D], bf16, tag="xlo")
                    nc.vector.tensor_tensor(out=xlo[:], in0=X, in1=xhi,
                                            op=Alu.subtract)
                    xsq = sqp.tile([P, D], bf16, tag="xsq")
                    nc.scalar.activation(xsq[:], X, Act.Square)
                    nc.tensor.matmul(out=ps_sums[seg][:], lhsT=oh[:],
                                     rhs=xhi, start=st, stop=False)
                    nc.tensor.matmul(out=ps_sums[seg][:], lhsT=oh[:],
                                     rhs=xlo[:], start=False, stop=sp)
                    nc.tensor.matmul(out=ps_sq[seg][:], lhsT=oh[:],
                                     rhs=xsq[:], start=st, stop=sp)
                    if sp:
                        seg_post(seg)
                t += L

            nc.sync.dma_start(out=part[:], in_=out4[:])

    nc.compile()
    return nc


def _prepare(feats, labels, demog):
    """Shard rows by (demog, label-half); sort each shard into two
    label-quarter segments padded to whole tiles. Returns the compile key,
    per-core input maps, and per-(core, seg, slot) counts for host combine.
    """
    lab256 = labels % 256
    core_id = demog * 2 + (labels >= 256).astype(np.int32)
    seg_id = (lab256 >= 128).astype(np.int32)
    labloc = (lab256 % 128).astype(np.float32)

    idx = [[np.flatnonzero((core_id == k) & (seg_id == s)) for s in range(2)]
           for k in range(N_CORES)]
    T0 = max(1, max(-(-len(idx[k][0]) // P) for k in range(N_CORES)))
    T1 = max(1, max(-(-len(idx[k][1]) // P) for k in range(N_CORES)))
    T = T0 + T1
    S = T * P

    iota_np = np.broadcast_to(np.arange(P, dtype=np.float32), (P, P))
    iota_np = np.ascontiguousarray(iota_np).astype(ml_dtypes.bfloat16)
    in_maps = []
    cnts = np.zeros((N_CORES, 2, P), np.int64)
    for k in range(N_CORES):
        f = np.zeros((S, D), np.float32)
        lab = np.full(S, PAD_LABEL, np.float32)
        for s, base in ((0, 0), (1, T0 * P)):
            rows = idx[k][s]
            f[base:base + len(rows)] = feats[rows]
            lab[base:base + len(rows)] = labloc[rows]
            cnts[k, s] = np.bincount(labloc[rows].astype(np.int64),
                                     minlength=P)
        labs_t = np.ascontiguousarray(lab.reshape(T, P).T)
        in_maps.append({"feats": f, "labels_t": labs_t, "iota_t": iota_np})
    return (T0, T1), in_maps, cnts


def _combine(parts, cnts):
    """Finish the reduction on host in fp64: parts[k] = [128,4] device out."""
    num = np.zeros(ND, np.float64)
    den = np.zeros(ND, np.float64)
    for k in range(N_CORES):
        d = k // 2
        p = np.asarray(parts[k], np.float64)
        for s in range(2):
            ssg = p[:, 2 * s]          # per-group sum of ||x||^2
            nn2 = p[:, 2 * s + 1]      # per-group ||sums||^2
            c = cnts[k, s].astype(np.float64)
            safe = np.maximum(c, 1.0)
            grp = (ssg - nn2 / safe) / safe
            pres = (c > 0)
            num[d] += grp[pres].sum()
            den[d] += pres.sum()
    intra = num / np.maximum(den, 1.0)
    return np.float32(np.mean(np.abs(intra - intra.mean())))


def kernel(feats, labels, demog_labels, _results_out=None):
    feats = np.ascontiguousarray(np.asarray(feats), dtype=np.float32)
    labels = np.asarray(labels).astype(np.int32)
    demog = np.asarray(demog_labels).astype(np.int32)
    assert feats.ndim == 2 and feats.shape[1] == D

    key, in_maps, cnts = _prepare(feats, labels, demog)
    nc = _cache.get(key)
    if nc is None:
        nc = _cache.setdefault(key, _build(*key))
    res = None
    last_exc = None
    for attempt in range(3):
        try:
            res = bass_utils.run_bass_kernel_spmd(
                nc, in_maps, core_ids=list(range(N_CORES)))
            break
        except Exception as e:  # transient axon worker hangups
            last_exc = e
            import time
            time.sleep(10)
    if res is None:
        raise last_exc
    if _results_out is not None:
        _results_out.append(res)
    return _combine([res.results[k]["part"] for k in range(N_CORES)], cnts)


# revision 10
# speedup vs baseline: 2.3705x; 1.0664x over previous
"""Trainium2 Bass kernel for nn_DebiasIntraDist (segment_reduce).

Full-input contract: kernel(**inputs) takes the complete (unsharded) inputs
and returns the full scalar loss. Sharding: core 2d+h gets the rows with
demog == d and label-half h, so every core owns a disjoint set of 256
(demog, label) groups. Within a core, rows are partitioned into two
segments by label-quarter (local label < 128 vs >= 128), each padded to a
whole number of 128-row tiles (T0, T1 known at compile time). Every tile
therefore feeds exactly ONE 128-group PSUM accumulator - half the matmul
work of an unsorted layout.

Per 128-row tile (single pass over feats, bf16 matmuls, error-compensated
hi/lo split so group sums are near-fp32 exact):
    oh   = one_hot(labels_local)  # vector IS_EQ, all-bf16 fast path
    xlo  = X - hi16(X)            # vector, bf16 out (hi16 = free strided view)
    xsq  = X * X                  # scalar engine Square, bf16 out
    sums[seg]  += oh^T @ hi + oh^T @ xlo    # tensor (one PSUM bank per seg)
    sumsq[seg] += oh^T @ xsq                # tensor
After each segment: norm2[g] = sum_d sums[g,d]^2 (scalar Square+accum) and
sumsq_g[g] = sum_d sumsq[g,d] (vector reduce) -> out tile [128, 4].

Feats are shipped partition-major ([P, T*D], rows t*128+p at column t*D)
so every DMA descriptor moves a contiguous 12 KiB run. iota and labels go
in as bf16 so the one-hot IS_EQ takes the 16-bit DVE fast path and the
loop can start as soon as the first feats tile lands - no gpsimd anywhere.

Each core DMAs its [128, 4] partial stats out; the host (which already
knows the per-group counts from the shard step) finishes the tiny O(G)
reduction to the scalar loss in fp64. No collectives anywhere.
"""

import os
import numpy as np
import ml_dtypes

try:
    import concourse.bacc as bacc
except ImportError:  # fresh environment without PYTHONPATH set up
    import sys
    for p in ("/root/.axon_site/_ro/trn_rl_repo", "/opt/trn_rl_repo",
              "/root/.axon_site/_ro/pypackages"):
        if p not in sys.path:
            sys.path.append(p)
    import concourse.bacc as bacc
import concourse.mybir as mybir
import concourse.tile as tile
import concourse.bass_utils as bass_utils

N_CORES = 8
P = 128
D = 512          # feature dim
ND = 4           # demog values
CH = 1           # per-tile feats DMA: tile-granular deps, no chunk stalls
PAD_LABEL = 500.0  # exact in bf16, never matches iota [0,128)

_cache: dict[tuple, object] = {}


def _build(T0: int, T1: int):
    """Compile the SPMD kernel: T0/T1 = tiles in segment 0/1."""
    T = T0 + T1
    fp32 = mybir.dt.float32
    bf16 = mybir.dt.bfloat16
    Alu = mybir.AluOpType
    Act = mybir.ActivationFunctionType

    nc = bacc.Bacc("TRN2", target_bir_lowering=False, debug=False,
                   enable_asserts=True, num_devices=N_CORES)

    feats = nc.dram_tensor("feats", [T * P, D], fp32,
                           kind="ExternalInput").ap()
    labels_t = nc.dram_tensor("labels_t", [P, T], fp32,
                              kind="ExternalInput").ap()
    iota_t = nc.dram_tensor("iota_t", [P, P], bf16,
                            kind="ExternalInput").ap()
    part = nc.dram_tensor("part", [P, 4], fp32, kind="ExternalOutput").ap()

    with tile.TileContext(nc) as tc:
        with (
            tc.tile_pool(name="const", bufs=1) as constp,
            tc.tile_pool(name="fx", bufs=10) as fxp,
            tc.tile_pool(name="oh", bufs=8) as ohp,
            tc.tile_pool(name="sq", bufs=6) as sqp,
            tc.tile_pool(name="hl", bufs=6) as hlp,
            tc.tile_pool(name="post", bufs=1) as postp,
            tc.tile_pool(name="ps", bufs=1, space="PSUM") as psp,
        ):
            # constants via DMA only - ready as soon as the queues go live
            labs = constp.tile([P, T], fp32, tag="labs")
            nc.scalar.dma_start(out=labs[:], in_=labels_t[:])
            iota = constp.tile([P, P], bf16, tag="iota")
            nc.scalar.dma_start(out=iota[:], in_=iota_t[:])

            # warm the activation table before the first real Square needs it
            warm = constp.tile([P, 1], fp32, tag="warm")
            nc.vector.memset(warm[:], 0.0)
            nc.scalar.activation(warm[:], warm[:], Act.Square)

            # per-segment accumulators: one PSUM bank each
            ps_sums = [psp.tile([P, D], fp32, tag=f"sums{s}", name=f"sums{s}")
                       for s in range(2)]
            ps_sq = [psp.tile([P, D], fp32, tag=f"sq{s}", name=f"sq{s}")
                     for s in range(2)]
            out4 = postp.tile([P, 4], fp32, tag="out4")
            feats_r = feats.rearrange("(n p) d -> n p d", p=P)  # [T, P, D]

            def seg_post(seg):
                # norm2[g] = sum_d sums[g,d]^2 ; sumsq_g[g] = sum_d sumsq[g,d]
                scr = postp.tile([P, D], fp32, tag=f"scr{seg}",
                                 name=f"scr{seg}")
                nc.scalar.activation(
                    scr[:], ps_sums[seg][:], Act.Square,
                    accum_out=out4[:, 2 * seg + 1:2 * seg + 2])
                nc.vector.tensor_reduce(
                    out=out4[:, 2 * seg:2 * seg + 1], in_=ps_sq[seg][:],
                    axis=mybir.AxisListType.X, op=Alu.add)

            t = 0
            first_chunk = True
            while t < T:
                # small first chunk so compute starts ASAP
                L = 1 if first_chunk else min(CH, T - t)
                first_chunk = False
                fx = fxp.tile([P, CH, D], fp32, tag="fx")
                nc.sync.dma_start(
                    out=fx[:, :L, :],
                    in_=feats_r[t:t + L].rearrange("n p d -> p n d"))
                fxb = fx[:].bitcast(bf16)  # [P, CH, 2D] for the hi view
                for j in range(L):
                    ti = t + j
                    seg = 0 if ti < T0 else 1
                    st = ti in (0, T0)
                    sp = ti in (T0 - 1, T - 1)
                    X = fx[:, j, :]
                    oh = ohp.tile([P, P], bf16, tag="oh")
                    nc.vector.tensor_scalar(
                        out=oh[:], in0=iota[:], scalar1=labs[:, ti:ti + 1],
                        scalar2=None, op0=Alu.is_equal,
                    )
                    xhi = fxb[:, j, 1::2]
                    xlo = hlp.tile([P, D], bf16, tag="xlo")
                    nc.vector.tensor_tensor(out=xlo[:], in0=X, in1=xhi,
                                            op=Alu.subtract)
                    xsq = sqp.tile([P, D], bf16, tag="xsq")
                    nc.scalar.activation(xsq[:], X, Act.Square)
                    nc.tensor.matmul(out=ps_sums[seg][:], lhsT=oh[:],
                                     rhs=xhi, start=st, stop=False)
                    nc.tensor.matmul(out=ps_sums[seg][:], lhsT=oh[:],
                                     rhs=xlo[:], start=False, stop=sp)
                    nc.tensor.matmul(out=ps_sq[seg][:], lhsT=oh[:],
                                     rhs=xsq[:], start=st, stop=sp)
                    if sp:
                        seg_post(seg)
                t += L

            nc.sync.dma_start(out=part[:], in_=out4[:])

    nc.compile()
    return nc


def _prepare(feats, labels, demog):
    """Shard rows by (demog, label-half); sort each shard into two
    label-quarter segments padded to whole tiles. Returns the compile key,
    per-core input maps, and per-(core, seg, slot) counts for host combine.
    """
    lab256 = labels % 256
    core_id = demog * 2 + (labels >= 256).astype(np.int32)
    seg_id = (lab256 >= 128).astype(np.int32)
    labloc = (lab256 % 128).astype(np.float32)

    idx = [[np.flatnonzero((core_id == k) & (seg_id == s)) for s in range(2)]
           for k in range(N_CORES)]
    T0 = max(1, max(-(-len(idx[k][0]) // P) for k in range(N_CORES)))
    T1 = max(1, max(-(-len(idx[k][1]) // P) for k in range(N_CORES)))
    T = T0 + T1
    S = T * P

    iota_np = np.broadcast_to(np.arange(P, dtype=np.float32), (P, P))
    iota_np = np.ascontiguousarray(iota_np).astype(ml_dtypes.bfloat16)
    in_maps = []
    cnts = np.zeros((N_CORES, 2, P), np.int64)
    for k in range(N_CORES):
        f = np.zeros((S, D), np.float32)
        lab = np.full(S, PAD_LABEL, np.float32)
        for s, base in ((0, 0), (1, T0 * P)):
            rows = idx[k][s]
            f[base:base + len(rows)] = feats[rows]
            lab[base:base + len(rows)] = labloc[rows]
            cnts[k, s] = np.bincount(labloc[rows].astype(np.int64),
                                     minlength=P)
        labs_t = np.ascontiguousarray(lab.reshape(T, P).T)
        in_maps.append({"feats": f, "labels_t": labs_t, "iota_t": iota_np})
    return (T0, T1), in_maps, cnts


def _combine(parts, cnts):
    """Finish the reduction on host in fp64: parts[k] = [128,4] device out."""
    num = np.zeros(ND, np.float64)
    den = np.zeros(ND, np.float64)
    for k in range(N_CORES):
        d = k // 2
        p = np.asarray(parts[k], np.float64)
        for s in range(2):
            ssg = p[:, 2 * s]          # per-group sum of ||x||^2
            nn2 = p[:, 2 * s + 1]      # per-group ||sums||^2
            c = cnts[k, s].astype(np.float64)
            safe = np.maximum(c, 1.0)
            grp = (ssg - nn2 / safe) / safe
            pres = (c > 0)
            num[d] += grp[pres].sum()
            den[d] += pres.sum()
    intra = num / np.maximum(den, 1.0)
    return np.float32(np.mean(np.abs(intra - intra.mean())))


def kernel(feats, labels, demog_labels, _results_out=None):
    feats = np.ascontiguousarray(np.asarray(feats), dtype=np.float32)
    labels = np.asarray(labels).astype(np.int32)
    demog = np.asarray(demog_labels).astype(np.int32)
    assert feats.ndim == 2 and feats.shape[1] == D

    key, in_maps, cnts = _prepare(feats, labels, demog)
    nc = _cache.get(key)
    if nc is None:
        nc = _cache.setdefault(key, _build(*key))
    res = None
    last_exc = None
    for attempt in range(3):
        try:
            res = bass_utils.run_bass_kernel_spmd(
                nc, in_maps, core_ids=list(range(N_CORES)))
            break
        except Exception as e:  # transient axon worker hangups
            last_exc = e
            import time
            time.sleep(10)
    if res is None:
        raise last_exc
    if _results_out is not None:
        _results_out.append(res)
    return _combine([res.results[k]["part"] for k in range(N_CORES)], cnts)
